# revision 6
# baseline (speedup 1.0000x reference)
"""GPT-J attention (B=1, S=2048, D=4096, H=16, HD=256, rot=64) on 8 TRN2 cores.

Strategy: tensor-parallel over heads (2 heads/core). Wq/Wk/Wv column-sharded,
Wo row-sharded, on-device ReduceScatter of the partial outputs; the host
concatenates the 8 [256, 4096] shards into the full [2048, 4096] output.

Host-side prep (cheap, numpy):
  - hsT = hidden_states.T so all device matmuls contract over the partition dim
  - per-core weight shards pre-transposed; Wq/Wk rows permuted within each head
    (even rot dims, odd rot dims, rest) so rotary becomes plain block ops
  - 1/sqrt(HD) folded into Wq; sin/cos tables and causal mask tiles precomputed

All device matmuls run as float32r (full-rate fp32 on the PE at N=512).
"""

import numpy as np

import concourse.bass as bass
import concourse.tile as tile
import concourse.mybir as mybir
from concourse import bacc
from concourse.bass_utils import run_bass_kernel_spmd

B, S, D = 1, 2048, 4096
H, HD, ROT = 16, 256, 64
NCORES = 8
HL = H // NCORES          # heads per core = 2
EL = D // NCORES          # local e width = 512
HALFW = S // 2            # 1024
P = 128
NROT2 = ROT // 2          # 32

f32 = mybir.dt.float32
f32r = mybir.dt.float32r
EXP = mybir.ActivationFunctionType.Exp
ADD = mybir.AluOpType.add
MUL = mybir.AluOpType.mult
SUB = mybir.AluOpType.subtract

_CACHE = {}


def _emit(nc, t):
    """Emit the whole per-core program inside a TileContext."""
    with tile.TileContext(nc) as tc:
        # ---------- global pools (live whole kernel) ----------
        with tc.tile_pool(name="const", bufs=1) as const_pool, \
             tc.tile_pool(name="wpan", bufs=4) as wpan_pool, \
             tc.tile_pool(name="stage", bufs=6) as stage_pool:

            ones_f = const_pool.tile([P, 1], f32)
            nc.vector.memset(ones_f[:], 1.0)
            ones_col = const_pool.tile([P, 1], f32r)
            nc.vector.tensor_copy(ones_col[:], ones_f[:])
            ones_rf = const_pool.tile([1, P], f32)
            nc.vector.memset(ones_rf[:], 1.0)
            ones_row = const_pool.tile([1, P], f32r)
            nc.vector.tensor_copy(ones_row[:], ones_rf[:])
            cos_sb = const_pool.tile([NROT2, S], f32)
            nc.sync.dma_start(out=cos_sb[:], in_=t["cosT"][:])
            sin_sb = const_pool.tile([NROT2, S], f32)
            nc.sync.dma_start(out=sin_sb[:], in_=t["sinT"][:])
            mask_sb = const_pool.tile([P, 4, 512], f32)
            nc.sync.dma_start(out=mask_sb[:], in_=t["masks"][:])

            # ================= Phase P: QKV projections =================
            with tc.tile_pool(name="hst", bufs=33) as hst_pool, \
                 tc.tile_pool(name="rot_scr", bufs=2) as rot_pool, \
                 tc.tile_pool(name="psum_p", bufs=8, space="PSUM") as psum_p:

                for half in range(2):
                    c0 = half * HALFW
                    hst = []
                    for dt in range(32):
                        ht = hst_pool.tile([P, HALFW], f32r, tag="hst")
                        nc.sync.dma_start(
                            out=ht[:], in_=t["hsT"][dt * P:(dt + 1) * P, c0:c0 + HALFW])
                        hst.append(ht)

                    def rot_evict(ps, stg, cols):
                        # partitions [0:32) even pairs, [32:64) odd pairs, rest plain
                        ca = cos_sb[:, cols:cols + 512]
                        sa = sin_sb[:, cols:cols + 512]
                        s1 = rot_pool.tile([NROT2, 512], f32, tag="rs1")
                        s2 = rot_pool.tile([NROT2, 512], f32, tag="rs2")
                        nc.vector.tensor_tensor(s1[:], ps[0:NROT2, :], ca, MUL)
                        nc.vector.tensor_tensor(s2[:], ps[NROT2:ROT, :], sa, MUL)
                        nc.vector.tensor_tensor(stg[0:NROT2, :], s1[:], s2[:], SUB)
                        s3 = rot_pool.tile([NROT2, 512], f32, tag="rs1")
                        s4 = rot_pool.tile([NROT2, 512], f32, tag="rs2")
                        nc.vector.tensor_tensor(s3[:], ps[NROT2:ROT, :], ca, MUL)
                        nc.vector.tensor_tensor(s4[:], ps[0:NROT2, :], sa, MUL)
                        nc.vector.tensor_tensor(stg[NROT2:ROT, :], s3[:], s4[:], ADD)
                        nc.vector.tensor_copy(stg[ROT:P, :], ps[ROT:P, :])

                    # ---- Q then K: out layout [e, s] ----
                    for wname, dest, do_rot in (("wqT", "qT_d", True),
                                                ("wkT", "kT_d", True)):
                        acc = [psum_p.tile([P, 512], f32, tag="pp", name="pp") for _ in range(8)]
                        for dt in range(32):
                            wp = wpan_pool.tile([P, EL], f32r, tag="wp")
                            nc.gpsimd.dma_start(
                                out=wp[:], in_=t[wname][dt * P:(dt + 1) * P, :])
                            for et in range(4):
                                for sc in range(2):
                                    nc.tensor.matmul(
                                        acc[et * 2 + sc][:],
                                        wp[:, et * P:(et + 1) * P],
                                        hst[dt][:, sc * 512:(sc + 1) * 512],
                                        start=(dt == 0), stop=(dt == 31))
                        for et in range(4):
                            for sc in range(2):
                                cols = c0 + sc * 512
                                stg = stage_pool.tile([P, 512], f32r, tag="stg")
                                ps = acc[et * 2 + sc]
                                if do_rot and et in (0, 2):
                                    rot_evict(ps, stg, cols)
                                else:
                                    nc.vector.tensor_copy(stg[:], ps[:])
                                nc.gpsimd.dma_start(
                                    out=t[dest][et * P:(et + 1) * P, cols:cols + 512],
                                    in_=stg[:])

                    # ---- V: natural layout [s, e] ----
                    acc = [psum_p.tile([P, 512], f32, tag="pp", name="pp") for _ in range(8)]
                    for dt in range(32):
                        wp = wpan_pool.tile([P, EL], f32r, tag="wp")
                        nc.gpsimd.dma_start(
                            out=wp[:], in_=t["wvT"][dt * P:(dt + 1) * P, :])
                        for st in range(8):
                            nc.tensor.matmul(
                                acc[st][:],
                                hst[dt][:, st * P:(st + 1) * P],
                                wp[:],
                                start=(dt == 0), stop=(dt == 31))
                    for st in range(8):
                        stg = stage_pool.tile([P, 512], f32r, tag="stg")
                        nc.vector.tensor_copy(stg[:], acc[st][:])
                        nc.gpsimd.dma_start(
                            out=t["v_d"][c0 + st * P:c0 + (st + 1) * P, :],
                            in_=stg[:])

            # ================= Phase A: attention =================
            with tc.tile_pool(name="kt", bufs=4) as kt_pool, \
                 tc.tile_pool(name="vh", bufs=18) as vh_pool, \
                 tc.tile_pool(name="qt", bufs=6) as qt_pool, \
                 tc.tile_pool(name="pt", bufs=6) as pt_pool, \
                 tc.tile_pool(name="cstg", bufs=4) as cstg_pool, \
                 tc.tile_pool(name="rcp", bufs=4) as rcp_pool, \
                 tc.tile_pool(name="ps_s", bufs=2, space="PSUM") as psum_s, \
                 tc.tile_pool(name="ps_c", bufs=4, space="PSUM") as psum_c, \
                 tc.tile_pool(name="ps_u", bufs=1, space="PSUM") as psum_u, \
                 tc.tile_pool(name="ps_b", bufs=1, space="PSUM") as psum_b:

                for h in range(HL):
                    e0 = h * HD
                    kt = []
                    for half in range(2):
                        ktile = kt_pool.tile([P, S], f32r, tag="kt")
                        nc.sync.dma_start(
                            out=ktile[:], in_=t["kT_d"][e0 + half * P:e0 + (half + 1) * P, :])
                        kt.append(ktile)
                    vh = []
                    for st in range(16):
                        vtile = vh_pool.tile([P, HD], f32r, tag="vh")
                        nc.sync.dma_start(
                            out=vtile[:],
                            in_=t["v_d"][st * P:(st + 1) * P, e0:e0 + HD])
                        vh.append(vtile)

                    for qg in range(4):
                        qt = []
                        for half in range(2):
                            qtile = qt_pool.tile([P, 512], f32r, tag="qt")
                            nc.sync.dma_start(
                                out=qtile[:],
                                in_=t["qT_d"][e0 + half * P:e0 + (half + 1) * P,
                                              qg * 512:(qg + 1) * 512])
                            qt.append(qtile)
                        ctxp = [psum_c.tile([P, 512], f32, tag="pc", name="pc") for _ in range(2)]
                        sump = psum_u.tile([1, 512], f32, tag="pu")
                        nkb = 4 * qg + 4
                        ps_list = [None] * nkb
                        p_list = [None] * nkb

                        def emit_scores(kb):
                            ps = psum_s.tile([P, 512], f32, tag="ps")
                            for half in range(2):
                                nc.tensor.matmul(
                                    ps[:],
                                    kt[half][:, kb * P:(kb + 1) * P],
                                    qt[half][:],
                                    start=(half == 0), stop=(half == 1))
                            ps_list[kb] = ps

                        emit_scores(0)
                        for kb in range(nkb):
                            ps = ps_list[kb]
                            if kb >= 4 * qg:
                                m = kb - 4 * qg
                                nc.vector.tensor_tensor(
                                    ps[:], ps[:], mask_sb[:, m, :], ADD)
                            p = pt_pool.tile([P, 512], f32r, tag="pt")
                            nc.scalar.activation(p[:], ps[:], EXP)
                            p_list[kb] = p
                            if kb + 1 < nkb:
                                emit_scores(kb + 1)
                            nc.tensor.matmul(
                                sump[:], ones_col[:], p[:],
                                start=(kb == 0), stop=(kb == nkb - 1))
                            for half in range(2):
                                nc.tensor.matmul(
                                    ctxp[half][:],
                                    vh[kb][:, half * P:(half + 1) * P],
                                    p[:],
                                    start=(kb == 0), stop=(kb == nkb - 1))

                        recip = rcp_pool.tile([1, 512], f32r, tag="rcp")
                        with nc.allow_low_precision(reason="f32r is full-width fp32 storage"):
                            nc.vector.reciprocal(recip[:], sump[:])
                        bb = psum_b.tile([P, 512], f32, tag="pb")
                        nc.tensor.matmul(bb[:], ones_row[:], recip[:],
                                         start=True, stop=True)
                        bb_sb = cstg_pool.tile([P, 512], f32, tag="bbsb")
                        nc.vector.tensor_copy(bb_sb[:], bb[:])
                        for half in range(2):
                            cst = cstg_pool.tile([P, 512], f32r, tag="cstg")
                            nc.vector.tensor_tensor(cst[:], ctxp[half][:], bb_sb[:], MUL)
                            et = h * 2 + half
                            nc.gpsimd.dma_start(
                                out=t["ctxT_d"][et * P:(et + 1) * P,
                                                qg * 512:(qg + 1) * 512],
                                in_=cst[:])

            # ================= Phase O: out projection =================
            with tc.tile_pool(name="octx", bufs=4) as octx_pool, \
                 tc.tile_pool(name="ps_o", bufs=4, space="PSUM") as psum_o:
                ctx_sb = []
                for et in range(4):
                    ct = octx_pool.tile([P, S], f32r, tag="octx")
                    nc.sync.dma_start(out=ct[:], in_=t["ctxT_d"][et * P:(et + 1) * P, :])
                    ctx_sb.append(ct)
                for fg in range(8):
                    wo = []
                    for et in range(4):
                        wt = wpan_pool.tile([P, 512], f32r, tag="wp")
                        nc.sync.dma_start(
                            out=wt[:],
                            in_=t["woT"][et * P:(et + 1) * P, fg * 512:(fg + 1) * 512])
                        wo.append(wt)
                    for st in range(16):
                        po = psum_o.tile([P, 512], f32, tag="po")
                        for et in range(4):
                            nc.tensor.matmul(
                                po[:],
                                ctx_sb[et][:, st * P:(st + 1) * P],
                                wo[et][:],
                                start=(et == 0), stop=(et == 3))
                        stg = stage_pool.tile([P, 512], f32, tag="stg")
                        nc.vector.tensor_copy(stg[:], po[:])
                        nc.gpsimd.dma_start(
                            out=t["pout_d"][st * P:(st + 1) * P,
                                            fg * 512:(fg + 1) * 512],
                            in_=stg[:])

            # ================= Phase R: reduce-scatter =================
            nc.gpsimd.collective_compute(
                "ReduceScatter",
                ADD,
                replica_groups=[list(range(NCORES))],
                ins=[t["pout_d"][:]],
                outs=[t["rs_d"][:]],
            )
            nc.sync.dma_start(out=t["out"][:], in_=t["rs_d"][:])


def _build():
    if "nc" in _CACHE:
        return _CACHE["nc"]
    nc = bacc.Bacc(None, num_devices=NCORES)
    t = {}
    t["hsT"] = nc.declare_dram_parameter("hsT", [D, S], f32r, isOutput=False)
    t["wqT"] = nc.declare_dram_parameter("wqT", [D, EL], f32r, isOutput=False)
    t["wkT"] = nc.declare_dram_parameter("wkT", [D, EL], f32r, isOutput=False)
    t["wvT"] = nc.declare_dram_parameter("wvT", [D, EL], f32r, isOutput=False)
    t["woT"] = nc.declare_dram_parameter("woT", [EL, D], f32r, isOutput=False)
    t["cosT"] = nc.declare_dram_parameter("cosT", [NROT2, S], f32, isOutput=False)
    t["sinT"] = nc.declare_dram_parameter("sinT", [NROT2, S], f32, isOutput=False)
    t["masks"] = nc.declare_dram_parameter("masks", [P, 4, 512], f32, isOutput=False)
    t["out"] = nc.declare_dram_parameter("out", [S // NCORES, D], f32, isOutput=True)
    t["qT_d"] = nc.dram_tensor("qT_d", [EL, S], f32r)
    t["kT_d"] = nc.dram_tensor("kT_d", [EL, S], f32r)
    t["v_d"] = nc.dram_tensor("v_d", [S, EL], f32r)
    t["ctxT_d"] = nc.dram_tensor("ctxT_d", [EL, S], f32r)
    t["pout_d"] = nc.dram_tensor("pout_d", [S, D], f32)
    t["rs_d"] = nc.dram_tensor("rs_d", [S // NCORES, D], f32)
    _emit(nc, t)
    nc.compile()
    _CACHE["nc"] = nc
    return nc


def _prep_inputs(hidden_states, Wq, Wk, Wv, Wo, attention_mask, position_ids):
    hs = np.asarray(hidden_states, np.float32).reshape(S, D)
    hsT = np.ascontiguousarray(hs.T)

    pos = np.asarray(position_ids).reshape(S).astype(np.float32)
    inv = 10000.0 ** (-np.arange(0, ROT, 2, dtype=np.float32) / ROT)  # [32]
    ang = pos[:, None] * inv[None, :]                                  # [S, 32]
    cosT = np.ascontiguousarray(np.cos(ang).T).astype(np.float32)
    sinT = np.ascontiguousarray(np.sin(ang).T).astype(np.float32)

    am = np.asarray(attention_mask, np.float32).reshape(S, S)
    masks = np.empty((P, 4, 512), np.float32)
    for m in range(4):
        # transposed-score layout: mask[p, n] for k = m*128+p, q = n
        masks[:, m, :] = am[0:512, m * P:(m + 1) * P].T

    # within-head row permutation: even rot dims, odd rot dims, the rest
    perm1 = np.concatenate([np.arange(0, ROT, 2), np.arange(1, ROT, 2),
                            np.arange(ROT, HD)])
    perm = np.concatenate([perm1 + HD * j for j in range(HL)])

    Wq = np.asarray(Wq, np.float32)
    Wk = np.asarray(Wk, np.float32)
    Wv = np.asarray(Wv, np.float32)
    Wo = np.asarray(Wo, np.float32)
    scale = 1.0 / np.sqrt(np.float32(HD))

    in_maps = []
    for c in range(NCORES):
        rows = slice(c * EL, (c + 1) * EL)
        wq_c = Wq[rows][perm] * scale
        wk_c = Wk[rows][perm]
        wv_c = Wv[rows]
        in_maps.append({
            "hsT": hsT,
            "wqT": np.ascontiguousarray(wq_c.T),
            "wkT": np.ascontiguousarray(wk_c.T),
            "wvT": np.ascontiguousarray(wv_c.T),
            "woT": np.ascontiguousarray(Wo[:, rows].T),
            "cosT": cosT,
            "sinT": sinT,
            "masks": masks,
        })
    return in_maps


def run(inputs, trace=False):
    """Run on HW. Returns (full_output, BassKernelResults)."""
    nc = _build()
    in_maps = _prep_inputs(**inputs)
    res = run_bass_kernel_spmd(nc, in_maps, list(range(NCORES)), trace=trace)
    shards = [res.results[c]["out"] for c in range(NCORES)]
    full = np.concatenate(shards, axis=0).reshape(B, S, D).astype(np.float32)
    return full, res


def kernel(**inputs):
    full, _ = run(inputs, trace=False)
    return full


# revision 7
# speedup vs baseline: 1.3650x; 1.3650x over previous
"""GPT-J attention (B=1, S=2048, D=4096, H=16, HD=256, rot=64) on 8 TRN2 cores.

Strategy: tensor-parallel over heads (2 heads/core). Wq/Wk/Wv column-sharded,
Wo row-sharded, on-device chunked ReduceScatter of the partial outputs
overlapped with attention/out-projection; the host reassembles the full
[2048, 4096] output from the 8 per-core shards.

Host-side prep (cheap, numpy):
  - hsT = hidden_states.T so all device matmuls contract over the partition dim
  - per-core weight shards pre-transposed; Wq/Wk rows permuted within each head
    (even rot dims, odd rot dims, rest) so rotary becomes plain block ops
  - 1/sqrt(HD) folded into Wq; sin/cos tables and causal mask tiles precomputed

Matmul operands are bf16 (fp32r measured 2 cyc/row on HW; bf16 is 1), all
accumulation in fp32 PSUM; softmax exp input, masks, rotary and the softmax
normalization run in fp32.
"""

import numpy as np
import ml_dtypes

import concourse.bass as bass
import concourse.tile as tile
import concourse.mybir as mybir
from concourse import bacc
from concourse.bass_utils import run_bass_kernel_spmd

B, S, D = 1, 2048, 4096
H, HD, ROT = 16, 256, 64
NCORES = 8
HL = H // NCORES          # heads per core = 2
EL = D // NCORES          # local e width = 512
HALFW = S // 2            # 1024
P = 128
NROT2 = ROT // 2          # 32

f32 = mybir.dt.float32
bf16 = mybir.dt.bfloat16
EXP = mybir.ActivationFunctionType.Exp
ADD = mybir.AluOpType.add
MUL = mybir.AluOpType.mult
SUB = mybir.AluOpType.subtract

_CACHE = {}


def _emit(nc, t):
    """Emit the whole per-core program inside a TileContext."""
    with tile.TileContext(nc) as tc:
        # ---------- global pools (live whole kernel) ----------
        with tc.tile_pool(name="const", bufs=1) as const_pool, \
             tc.tile_pool(name="wpan", bufs=4) as wpan_pool, \
             tc.tile_pool(name="stage", bufs=6) as stage_pool:

            ones_col = const_pool.tile([P, 1], bf16)
            nc.vector.memset(ones_col[:], 1.0)
            ones_row = const_pool.tile([1, P], f32)
            nc.vector.memset(ones_row[:], 1.0)
            cos_sb = const_pool.tile([NROT2, S], f32)
            nc.sync.dma_start(out=cos_sb[:], in_=t["cosT"][:])
            sin_sb = const_pool.tile([NROT2, S], f32)
            nc.sync.dma_start(out=sin_sb[:], in_=t["sinT"][:])
            mask_sb = const_pool.tile([P, 4, 512], f32)
            nc.sync.dma_start(out=mask_sb[:], in_=t["masks"][:])

            # ================= Phase P: QKV projections =================
            with tc.tile_pool(name="hst", bufs=66) as hst_pool, \
                 tc.tile_pool(name="rot_scr", bufs=2) as rot_pool, \
                 tc.tile_pool(name="psum_p", bufs=8, space="PSUM") as psum_p:

                hst_all = [[None] * 32, [None] * 32]
                for half in range(2):
                    c0 = half * HALFW
                    for dt in range(32):
                        ht = hst_pool.tile([P, HALFW], bf16, tag="hst", name="hst")
                        nc.sync.dma_start(
                            out=ht[:], in_=t["hsT"][dt * P:(dt + 1) * P, c0:c0 + HALFW])
                        hst_all[half][dt] = ht

                def rot_evict(ps, stg, cols):
                    # partitions [0:32) even pairs, [32:64) odd pairs, rest plain
                    ca = cos_sb[:, cols:cols + 512]
                    sa = sin_sb[:, cols:cols + 512]
                    s1 = rot_pool.tile([NROT2, 512], f32, tag="rs1", name="rs1")
                    s2 = rot_pool.tile([NROT2, 512], f32, tag="rs2", name="rs2")
                    nc.vector.tensor_tensor(s1[:], ps[0:NROT2, :], ca, MUL)
                    nc.vector.tensor_tensor(s2[:], ps[NROT2:ROT, :], sa, MUL)
                    nc.vector.tensor_tensor(stg[0:NROT2, :], s1[:], s2[:], SUB)
                    s3 = rot_pool.tile([NROT2, 512], f32, tag="rs1", name="rs1")
                    s4 = rot_pool.tile([NROT2, 512], f32, tag="rs2", name="rs2")
                    nc.vector.tensor_tensor(s3[:], ps[NROT2:ROT, :], ca, MUL)
                    nc.vector.tensor_tensor(s4[:], ps[0:NROT2, :], sa, MUL)
                    nc.vector.tensor_tensor(stg[NROT2:ROT, :], s3[:], s4[:], ADD)
                    nc.vector.tensor_copy(stg[ROT:P, :], ps[ROT:P, :])

                for half in range(2):
                    c0 = half * HALFW
                    hst = hst_all[half]
                    # ---- Q then K: out layout [e, s] ----
                    for wname, dest, do_rot in (("wqT", "qT_d", True),
                                                ("wkT", "kT_d", True)):
                        acc = [psum_p.tile([P, 512], f32, tag="pp", name="pp")
                               for _ in range(8)]
                        for dt in range(32):
                            wp = wpan_pool.tile([P, EL], bf16, tag="wp", name="wp")
                            nc.gpsimd.dma_start(
                                out=wp[:], in_=t[wname][dt * P:(dt + 1) * P, :])
                            for et in range(4):
                                for sc in range(2):
                                    nc.tensor.matmul(
                                        acc[et * 2 + sc][:],
                                        wp[:, et * P:(et + 1) * P],
                                        hst[dt][:, sc * 512:(sc + 1) * 512],
                                        start=(dt == 0), stop=(dt == 31))
                        for et in range(4):
                            for sc in range(2):
                                cols = c0 + sc * 512
                                stg = stage_pool.tile([P, 512], bf16, tag="stg",
                                                      name="stg")
                                ps = acc[et * 2 + sc]
                                if do_rot and et in (0, 2):
                                    rot_evict(ps, stg, cols)
                                else:
                                    nc.vector.tensor_copy(stg[:], ps[:])
                                nc.gpsimd.dma_start(
                                    out=t[dest][et * P:(et + 1) * P, cols:cols + 512],
                                    in_=stg[:])

                    # ---- V: natural layout [s, e] ----
                    acc = [psum_p.tile([P, 512], f32, tag="pp", name="pp")
                           for _ in range(8)]
                    for dt in range(32):
                        wp = wpan_pool.tile([P, EL], bf16, tag="wp", name="wp")
                        nc.gpsimd.dma_start(
                            out=wp[:], in_=t["wvT"][dt * P:(dt + 1) * P, :])
                        for st in range(8):
                            nc.tensor.matmul(
                                acc[st][:],
                                hst[dt][:, st * P:(st + 1) * P],
                                wp[:],
                                start=(dt == 0), stop=(dt == 31))
                    for st in range(8):
                        stg = stage_pool.tile([P, 512], bf16, tag="stg", name="stg")
                        nc.vector.tensor_copy(stg[:], acc[st][:])
                        nc.gpsimd.dma_start(
                            out=t["v_d"][c0 + st * P:c0 + (st + 1) * P, :],
                            in_=stg[:])

            # ============ Phase A+O: attention + out-proj, chunked RS ============
            with tc.tile_pool(name="kt", bufs=4) as kt_pool, \
                 tc.tile_pool(name="vh", bufs=32) as vh_pool, \
                 tc.tile_pool(name="wot", bufs=4) as wot_pool, \
                 tc.tile_pool(name="qt", bufs=6) as qt_pool, \
                 tc.tile_pool(name="pt", bufs=6) as pt_pool, \
                 tc.tile_pool(name="cstg", bufs=8) as cstg_pool, \
                 tc.tile_pool(name="rcp", bufs=4) as rcp_pool, \
                 tc.tile_pool(name="ps_s", bufs=2, space="PSUM") as psum_s, \
                 tc.tile_pool(name="ps_c", bufs=4, space="PSUM") as psum_c, \
                 tc.tile_pool(name="ps_u", bufs=1, space="PSUM") as psum_u, \
                 tc.tile_pool(name="ps_b", bufs=1, space="PSUM") as psum_b:

                # resident K^T and V for both heads, resident Wo^T
                kt = [[None, None], [None, None]]
                vh = [[None] * 16, [None] * 16]
                for h in range(HL):
                    e0 = h * HD
                    for half in range(2):
                        ktile = kt_pool.tile([P, S], bf16, tag="kt", name="kt")
                        nc.sync.dma_start(
                            out=ktile[:],
                            in_=t["kT_d"][e0 + half * P:e0 + (half + 1) * P, :])
                        kt[h][half] = ktile
                    for st in range(16):
                        vtile = vh_pool.tile([P, HD], bf16, tag="vh", name="vh")
                        nc.sync.dma_start(
                            out=vtile[:],
                            in_=t["v_d"][st * P:(st + 1) * P, e0:e0 + HD])
                        vh[h][st] = vtile
                wot = []
                for et in range(4):
                    wtile = wot_pool.tile([P, D], bf16, tag="wot", name="wot")
                    nc.sync.dma_start(out=wtile[:],
                                      in_=t["woT"][et * P:(et + 1) * P, :])
                    wot.append(wtile)

                for qg in range(4):
                    cst_all = [None] * 4  # et -> [128, 512] bf16 ctx^T tile
                    for h in range(HL):
                        e0 = h * HD
                        qt = []
                        for half in range(2):
                            qtile = qt_pool.tile([P, 512], bf16, tag="qt", name="qt")
                            nc.sync.dma_start(
                                out=qtile[:],
                                in_=t["qT_d"][e0 + half * P:e0 + (half + 1) * P,
                                              qg * 512:(qg + 1) * 512])
                            qt.append(qtile)
                        ctxp = [psum_c.tile([P, 512], f32, tag="pc", name="pc")
                                for _ in range(2)]
                        sump = psum_u.tile([1, 512], f32, tag="pu", name="pu")
                        nkb = 4 * qg + 4
                        ps_list = [None] * nkb

                        def emit_scores(kb):
                            ps = psum_s.tile([P, 512], f32, tag="ps", name="ps")
                            for half in range(2):
                                nc.tensor.matmul(
                                    ps[:],
                                    kt[h][half][:, kb * P:(kb + 1) * P],
                                    qt[half][:],
                                    start=(half == 0), stop=(half == 1))
                            ps_list[kb] = ps

                        emit_scores(0)
                        for kb in range(nkb):
                            ps = ps_list[kb]
                            if kb >= 4 * qg:
                                m = kb - 4 * qg
                                nc.vector.tensor_tensor(
                                    ps[:], ps[:], mask_sb[:, m, :], ADD)
                            p = pt_pool.tile([P, 512], bf16, tag="pt", name="pt")
                            nc.scalar.activation(p[:], ps[:], EXP)
                            if kb + 1 < nkb:
                                emit_scores(kb + 1)
                            nc.tensor.matmul(
                                sump[:], ones_col[:], p[:],
                                start=(kb == 0), stop=(kb == nkb - 1))
                            for half in range(2):
                                nc.tensor.matmul(
                                    ctxp[half][:],
                                    vh[h][kb][:, half * P:(half + 1) * P],
                                    p[:],
                                    start=(kb == 0), stop=(kb == nkb - 1))

                        recip = rcp_pool.tile([1, 512], f32, tag="rcp", name="rcp")
                        nc.vector.reciprocal(recip[:], sump[:])
                        bb = psum_b.tile([P, 512], f32, tag="pb", name="pb")
                        nc.tensor.matmul(bb[:], ones_row[:], recip[:],
                                         start=True, stop=True)
                        bb_sb = cstg_pool.tile([P, 512], f32, tag="bbsb", name="bbsb")
                        nc.vector.tensor_copy(bb_sb[:], bb[:])
                        for half in range(2):
                            cst = cstg_pool.tile([P, 512], bf16, tag="cstg",
                                                 name="cstg")
                            nc.vector.tensor_tensor(cst[:], ctxp[half][:],
                                                    bb_sb[:], MUL)
                            cst_all[h * 2 + half] = cst

                    # ---- out-proj for rows [512*qg, 512*qg+512) ----
                    for st in range(4):
                        r0 = qg * 512 + st * P
                        for fg in range(8):
                            po = psum_s.tile([P, 512], f32, tag="ps", name="po")
                            for et in range(4):
                                nc.tensor.matmul(
                                    po[:],
                                    cst_all[et][:, st * P:(st + 1) * P],
                                    wot[et][:, fg * 512:(fg + 1) * 512],
                                    start=(et == 0), stop=(et == 3))
                            stg = stage_pool.tile([P, 512], bf16, tag="stg",
                                                  name="stg")
                            nc.vector.tensor_copy(stg[:], po[:])
                            nc.gpsimd.dma_start(
                                out=t["pout_d"][r0:r0 + P, fg * 512:(fg + 1) * 512],
                                in_=stg[:])

                    # ---- reduce-scatter this 512-row chunk ----
                    nc.gpsimd.collective_compute(
                        "ReduceScatter",
                        ADD,
                        replica_groups=[list(range(NCORES))],
                        ins=[t["pout_d"][qg * 512:(qg + 1) * 512, :]],
                        outs=[t["rs_d"][qg * 64:(qg + 1) * 64, :]],
                    )

            nc.sync.dma_start(out=t["out"][:], in_=t["rs_d"][:])


def _build():
    if "nc" in _CACHE:
        return _CACHE["nc"]
    nc = bacc.Bacc(None, num_devices=NCORES)
    t = {}
    t["hsT"] = nc.declare_dram_parameter("hsT", [D, S], bf16, isOutput=False)
    t["wqT"] = nc.declare_dram_parameter("wqT", [D, EL], bf16, isOutput=False)
    t["wkT"] = nc.declare_dram_parameter("wkT", [D, EL], bf16, isOutput=False)
    t["wvT"] = nc.declare_dram_parameter("wvT", [D, EL], bf16, isOutput=False)
    t["woT"] = nc.declare_dram_parameter("woT", [EL, D], bf16, isOutput=False)
    t["cosT"] = nc.declare_dram_parameter("cosT", [NROT2, S], f32, isOutput=False)
    t["sinT"] = nc.declare_dram_parameter("sinT", [NROT2, S], f32, isOutput=False)
    t["masks"] = nc.declare_dram_parameter("masks", [P, 4, 512], f32, isOutput=False)
    t["out"] = nc.declare_dram_parameter("out", [S // NCORES, D], bf16, isOutput=True)
    t["qT_d"] = nc.dram_tensor("qT_d", [EL, S], bf16)
    t["kT_d"] = nc.dram_tensor("kT_d", [EL, S], bf16)
    t["v_d"] = nc.dram_tensor("v_d", [S, EL], bf16)
    t["pout_d"] = nc.dram_tensor("pout_d", [S, D], bf16)
    t["rs_d"] = nc.dram_tensor("rs_d", [S // NCORES, D], bf16)
    _emit(nc, t)
    nc.compile()
    _CACHE["nc"] = nc
    return nc


def _prep_inputs(hidden_states, Wq, Wk, Wv, Wo, attention_mask, position_ids):
    hs = np.asarray(hidden_states, np.float32).reshape(S, D)
    hsT = np.ascontiguousarray(hs.T).astype(ml_dtypes.bfloat16)

    pos = np.asarray(position_ids).reshape(S).astype(np.float32)
    inv = 10000.0 ** (-np.arange(0, ROT, 2, dtype=np.float32) / ROT)  # [32]
    ang = pos[:, None] * inv[None, :]                                  # [S, 32]
    cosT = np.ascontiguousarray(np.cos(ang).T).astype(np.float32)
    sinT = np.ascontiguousarray(np.sin(ang).T).astype(np.float32)

    am = np.asarray(attention_mask, np.float32).reshape(S, S)
    masks = np.empty((P, 4, 512), np.float32)
    for m in range(4):
        # transposed-score layout: mask[p, n] for k = m*128+p, q = n
        masks[:, m, :] = am[0:512, m * P:(m + 1) * P].T

    # within-head row permutation: even rot dims, odd rot dims, the rest
    perm1 = np.concatenate([np.arange(0, ROT, 2), np.arange(1, ROT, 2),
                            np.arange(ROT, HD)])
    perm = np.concatenate([perm1 + HD * j for j in range(HL)])

    Wq = np.asarray(Wq, np.float32)
    Wk = np.asarray(Wk, np.float32)
    Wv = np.asarray(Wv, np.float32)
    Wo = np.asarray(Wo, np.float32)
    scale = 1.0 / np.sqrt(np.float32(HD))

    in_maps = []
    for c in range(NCORES):
        rows = slice(c * EL, (c + 1) * EL)
        wq_c = Wq[rows][perm] * scale
        wk_c = Wk[rows][perm]
        wv_c = Wv[rows]
        in_maps.append({
            "hsT": hsT,
            "wqT": np.ascontiguousarray(wq_c.T).astype(ml_dtypes.bfloat16),
            "wkT": np.ascontiguousarray(wk_c.T).astype(ml_dtypes.bfloat16),
            "wvT": np.ascontiguousarray(wv_c.T).astype(ml_dtypes.bfloat16),
            "woT": np.ascontiguousarray(Wo[:, rows].T).astype(ml_dtypes.bfloat16),
            "cosT": cosT,
            "sinT": sinT,
            "masks": masks,
        })
    return in_maps


def run(inputs, trace=False):
    """Run on HW. Returns (full_output, BassKernelResults)."""
    nc = _build()
    in_maps = _prep_inputs(**inputs)
    res = run_bass_kernel_spmd(nc, in_maps, list(range(NCORES)), trace=trace)
    # RS chunk j on core c holds global rows [512j + 64c, 512j + 64c + 64)
    full = np.empty((S, D), np.float32)
    for c in range(NCORES):
        shard = np.asarray(res.results[c]["out"]).astype(np.float32)
        for j in range(4):
            full[512 * j + 64 * c:512 * j + 64 * c + 64] = shard[64 * j:64 * (j + 1)]
    return full.reshape(B, S, D), res


def kernel(**inputs):
    full, _ = run(inputs, trace=False)
    return full


# revision 9
# speedup vs baseline: 1.4375x; 1.0532x over previous
"""GPT-J attention (B=1, S=2048, D=4096, H=16, HD=256, rot=64) on 8 TRN2 cores.

Strategy: tensor-parallel over heads (2 heads/core). Wq/Wk/Wv column-sharded,
Wo row-sharded, on-device chunked ReduceScatter of the partial outputs
overlapped with attention/out-projection; the host reassembles the full
[2048, 4096] output from the 8 per-core shards.

Host-side prep (cheap, numpy):
  - hsT = hidden_states.T so all device matmuls contract over the partition dim
  - per-core weight shards pre-transposed; Wq/Wk rows permuted within each head
    (even rot dims, odd rot dims, rest) so rotary becomes plain block ops
  - 1/sqrt(HD) folded into Wq; sin/cos tables and causal mask tiles precomputed

Matmul operands are bf16 (fp32r measured 2 cyc/row on HW; bf16 is 1), all
accumulation in fp32 PSUM; softmax exp input, masks, rotary and the softmax
normalization run in fp32. K^T, V and ctx^T stay SBUF-resident between phases;
only q roundtrips through DRAM. PSUM evictions alternate between DVE and ACT.
"""

import contextlib
import numpy as np
import ml_dtypes

import concourse.bass as bass
import concourse.tile as tile
import concourse.mybir as mybir
from concourse import bacc
from concourse.bass_utils import run_bass_kernel_spmd

B, S, D = 1, 2048, 4096
H, HD, ROT = 16, 256, 64
NCORES = 8
HL = H // NCORES          # heads per core = 2
EL = D // NCORES          # local e width = 512
HALFW = S // 2            # 1024
P = 128
NROT2 = ROT // 2          # 32

f32 = mybir.dt.float32
bf16 = mybir.dt.bfloat16
EXP = mybir.ActivationFunctionType.Exp
COPY = mybir.ActivationFunctionType.Copy
ADD = mybir.AluOpType.add
MUL = mybir.AluOpType.mult
SUB = mybir.AluOpType.subtract

# output row chunks: (global_row_start, nrows) per ReduceScatter call
RS_CHUNKS = [(0, 512), (512, 512), (1024, 512), (1536, 256), (1792, 256)]

_CACHE = {}


def _emit(nc, t):
    """Emit the whole per-core program inside a TileContext."""
    with tile.TileContext(nc) as tc:
        with contextlib.ExitStack() as _stk:
            ec = _stk.enter_context
            const_pool = ec(tc.tile_pool(name="const", bufs=1))
            wpan_pool = ec(tc.tile_pool(name="wpan", bufs=4))
            stage_pool = ec(tc.tile_pool(name="stage", bufs=4))
            hst_pool = ec(tc.tile_pool(name="hst", bufs=32))
            kres_pool = ec(tc.tile_pool(name="kres", bufs=16))
            vres_pool = ec(tc.tile_pool(name="vres", bufs=16))
            wot_pool = ec(tc.tile_pool(name="wot", bufs=4))
            rot_pool = ec(tc.tile_pool(name="rot_scr", bufs=1))
            qt_pool = ec(tc.tile_pool(name="qt", bufs=4))
            pt_pool = ec(tc.tile_pool(name="pt", bufs=4))
            bbsb_pool = ec(tc.tile_pool(name="bbsb", bufs=2))
            cstg_pool = ec(tc.tile_pool(name="cstg", bufs=8))
            rcp_pool = ec(tc.tile_pool(name="rcp", bufs=2))

            ones_col = const_pool.tile([P, 1], bf16)
            nc.vector.memset(ones_col[:], 1.0)
            ones_row = const_pool.tile([1, P], f32)
            nc.vector.memset(ones_row[:], 1.0)
            cos_sb = const_pool.tile([NROT2, S], f32)
            nc.sync.dma_start(out=cos_sb[:], in_=t["cosT"][:])
            sin_sb = const_pool.tile([NROT2, S], f32)
            nc.sync.dma_start(out=sin_sb[:], in_=t["sinT"][:])
            mask_sb = const_pool.tile([P, 4, 512], f32)
            nc.sync.dma_start(out=mask_sb[:], in_=t["masks"][:])

            # resident Wo^T (used by phase O, loaded early to overlap)
            wot = []
            for et in range(4):
                wtile = wot_pool.tile([P, D], bf16, tag="wot", name="wot")
                nc.sync.dma_start(out=wtile[:],
                                  in_=t["woT"][et * P:(et + 1) * P, :])
                wot.append(wtile)

            kres = [[None] * 4 for _ in range(4)]   # [et][scg] -> [128, 512]
            vres = [None] * 16                      # [st16]    -> [128, 512]

            def rot_evict(ps, stg, cols):
                # partitions [0:32) even pairs, [32:64) odd pairs, rest plain
                ca = cos_sb[:, cols:cols + 512]
                sa = sin_sb[:, cols:cols + 512]
                s1 = rot_pool.tile([NROT2, 512], f32, tag="rs1", name="rs1")
                s2 = rot_pool.tile([NROT2, 512], f32, tag="rs2", name="rs2")
                nc.vector.tensor_tensor(s1[:], ps[0:NROT2, :], ca, MUL)
                nc.vector.tensor_tensor(s2[:], ps[NROT2:ROT, :], sa, MUL)
                nc.vector.tensor_tensor(stg[0:NROT2, :], s1[:], s2[:], SUB)
                s3 = rot_pool.tile([NROT2, 512], f32, tag="rs1", name="rs1")
                s4 = rot_pool.tile([NROT2, 512], f32, tag="rs2", name="rs2")
                nc.vector.tensor_tensor(s3[:], ps[NROT2:ROT, :], ca, MUL)
                nc.vector.tensor_tensor(s4[:], ps[0:NROT2, :], sa, MUL)
                nc.vector.tensor_tensor(stg[NROT2:ROT, :], s3[:], s4[:], ADD)
                nc.scalar.activation(stg[ROT:P, :], ps[ROT:P, :], COPY)

            def evict(dst_ap, src_ps, on_act):
                if on_act:
                    nc.scalar.activation(dst_ap, src_ps, COPY)
                else:
                    nc.vector.tensor_copy(dst_ap, src_ps)

            # ================= Phase P: QKV projections =================
            with tc.tile_pool(name="psum_p", bufs=8, space="PSUM") as psum_p:
                for half in range(2):
                    c0 = half * HALFW
                    hst = []
                    for dt in range(32):
                        ht = hst_pool.tile([P, HALFW], bf16, tag="hst", name="hst")
                        nc.sync.dma_start(
                            out=ht[:], in_=t["hsT"][dt * P:(dt + 1) * P,
                                                    c0:c0 + HALFW])
                        hst.append(ht)

                    # ---- Q then K: out layout [e, s] ----
                    for wname, is_q in (("wqT", True), ("wkT", False)):
                        acc = [psum_p.tile([P, 512], f32, tag="pp", name="pp")
                               for _ in range(8)]
                        for dt in range(32):
                            wp = wpan_pool.tile([P, EL], bf16, tag="wp", name="wp")
                            nc.gpsimd.dma_start(
                                out=wp[:], in_=t[wname][dt * P:(dt + 1) * P, :])
                            for et in range(4):
                                for sc in range(2):
                                    nc.tensor.matmul(
                                        acc[et * 2 + sc][:],
                                        wp[:, et * P:(et + 1) * P],
                                        hst[dt][:, sc * 512:(sc + 1) * 512],
                                        start=(dt == 0), stop=(dt == 31))
                        for et in range(4):
                            for sc in range(2):
                                cols = c0 + sc * 512
                                ps = acc[et * 2 + sc]
                                if is_q:
                                    stg = stage_pool.tile([P, 512], bf16,
                                                          tag="stg", name="stg")
                                    if et in (0, 2):
                                        rot_evict(ps, stg, cols)
                                    else:
                                        evict(stg[:], ps[:], on_act=(sc == 1))
                                    nc.scalar.dma_start(
                                        out=t["qT_d"][et * P:(et + 1) * P,
                                                      cols:cols + 512],
                                        in_=stg[:])
                                else:
                                    ktile = kres_pool.tile([P, 512], bf16,
                                                           tag="kres", name="kres")
                                    if et in (0, 2):
                                        rot_evict(ps, ktile, cols)
                                    else:
                                        evict(ktile[:], ps[:], on_act=(sc == 1))
                                    kres[et][half * 2 + sc] = ktile

                    # ---- V: natural layout [s, e] ----
                    acc = [psum_p.tile([P, 512], f32, tag="pp", name="pp")
                           for _ in range(8)]
                    for dt in range(32):
                        wp = wpan_pool.tile([P, EL], bf16, tag="wp", name="wp")
                        nc.gpsimd.dma_start(
                            out=wp[:], in_=t["wvT"][dt * P:(dt + 1) * P, :])
                        for st in range(8):
                            nc.tensor.matmul(
                                acc[st][:],
                                hst[dt][:, st * P:(st + 1) * P],
                                wp[:],
                                start=(dt == 0), stop=(dt == 31))
                    for st in range(8):
                        vtile = vres_pool.tile([P, 512], bf16, tag="vres",
                                               name="vres")
                        evict(vtile[:], acc[st][:], on_act=(st % 2 == 1))
                        vres[half * 8 + st] = vtile

            # ============ Phase A+O: attention + out-proj, chunked RS ============
            with contextlib.ExitStack() as _stk2:
                ec2 = _stk2.enter_context
                psum_s = ec2(tc.tile_pool(name="ps_s", bufs=2, space="PSUM"))
                psum_c = ec2(tc.tile_pool(name="ps_c", bufs=4, space="PSUM"))
                psum_u = ec2(tc.tile_pool(name="ps_u", bufs=1, space="PSUM"))
                psum_b = ec2(tc.tile_pool(name="ps_b", bufs=1, space="PSUM"))

                rs_done = 0  # rows already reduce-scattered
                rs_idx = 0

                for qg in range(4):
                    cst_all = [None] * 4  # et -> [128, 512] bf16 ctx^T tile
                    for h in range(HL):
                        e0 = h * HD
                        qt = []
                        for half in range(2):
                            qtile = qt_pool.tile([P, 512], bf16, tag="qt",
                                                 name="qt")
                            nc.sync.dma_start(
                                out=qtile[:],
                                in_=t["qT_d"][e0 + half * P:e0 + (half + 1) * P,
                                              qg * 512:(qg + 1) * 512])
                            qt.append(qtile)
                        ctxp = [psum_c.tile([P, 512], f32, tag="pc", name="pc")
                                for _ in range(2)]
                        sump = psum_u.tile([1, 512], f32, tag="pu", name="pu")
                        nkb = 4 * qg + 4
                        ps_list = [None] * nkb

                        def emit_scores(kb):
                            ps = psum_s.tile([P, 512], f32, tag="ps", name="ps")
                            for half in range(2):
                                nc.tensor.matmul(
                                    ps[:],
                                    kres[h * 2 + half][kb // 4][
                                        :, (kb % 4) * P:(kb % 4 + 1) * P],
                                    qt[half][:],
                                    start=(half == 0), stop=(half == 1))
                            ps_list[kb] = ps

                        emit_scores(0)
                        for kb in range(nkb):
                            ps = ps_list[kb]
                            if kb >= 4 * qg:
                                m = kb - 4 * qg
                                nc.vector.tensor_tensor(
                                    ps[:], ps[:], mask_sb[:, m, :], ADD)
                            p = pt_pool.tile([P, 512], bf16, tag="pt", name="pt")
                            nc.scalar.activation(p[:], ps[:], EXP)
                            if kb + 1 < nkb:
                                emit_scores(kb + 1)
                            nc.tensor.matmul(
                                sump[:], ones_col[:], p[:],
                                start=(kb == 0), stop=(kb == nkb - 1))
                            for half in range(2):
                                nc.tensor.matmul(
                                    ctxp[half][:],
                                    vres[kb][:, e0 + half * P:e0 + (half + 1) * P],
                                    p[:],
                                    start=(kb == 0), stop=(kb == nkb - 1))

                        recip = rcp_pool.tile([1, 512], f32, tag="rcp", name="rcp")
                        nc.vector.reciprocal(recip[:], sump[:])
                        bb = psum_b.tile([P, 512], f32, tag="pb", name="pb")
                        nc.tensor.matmul(bb[:], ones_row[:], recip[:],
                                         start=True, stop=True)
                        bb_sb = bbsb_pool.tile([P, 512], f32, tag="bbsb",
                                               name="bbsb")
                        nc.vector.tensor_copy(bb_sb[:], bb[:])
                        for half in range(2):
                            cst = cstg_pool.tile([P, 512], bf16, tag="cstg",
                                                 name="cstg")
                            nc.vector.tensor_tensor(cst[:], ctxp[half][:],
                                                    bb_sb[:], MUL)
                            cst_all[h * 2 + half] = cst

                    # ---- out-proj for rows [512*qg, 512*qg+512) ----
                    for st in range(4):
                        r0 = qg * 512 + st * P
                        for fg in range(8):
                            po = psum_s.tile([P, 512], f32, tag="ps", name="po")
                            for et in range(4):
                                nc.tensor.matmul(
                                    po[:],
                                    cst_all[et][:, st * P:(st + 1) * P],
                                    wot[et][:, fg * 512:(fg + 1) * 512],
                                    start=(et == 0), stop=(et == 3))
                            stg = stage_pool.tile([P, 512], bf16, tag="stg",
                                                  name="stg")
                            evict(stg[:], po[:], on_act=(fg % 2 == 1))
                            nc.gpsimd.dma_start(
                                out=t["pout_d"][r0:r0 + P,
                                                fg * 512:(fg + 1) * 512],
                                in_=stg[:])

                        # fire any RS chunks fully covered by stored rows
                        rows_stored = qg * 512 + (st + 1) * P
                        while rs_idx < len(RS_CHUNKS):
                            start_r, nr = RS_CHUNKS[rs_idx]
                            if start_r + nr > rows_stored:
                                break
                            o0 = start_r // NCORES
                            nc.gpsimd.collective_compute(
                                "ReduceScatter",
                                ADD,
                                replica_groups=[list(range(NCORES))],
                                ins=[t["pout_d"][start_r:start_r + nr, :]],
                                outs=[t["rs_d"][o0:o0 + nr // NCORES, :]],
                            )
                            nc.sync.dma_start(
                                out=t["out"][o0:o0 + nr // NCORES, :],
                                in_=t["rs_d"][o0:o0 + nr // NCORES, :])
                            rs_idx += 1


def _build():
    if "nc" in _CACHE:
        return _CACHE["nc"]
    nc = bacc.Bacc(None, num_devices=NCORES)
    t = {}
    t["hsT"] = nc.declare_dram_parameter("hsT", [D, S], bf16, isOutput=False)
    t["wqT"] = nc.declare_dram_parameter("wqT", [D, EL], bf16, isOutput=False)
    t["wkT"] = nc.declare_dram_parameter("wkT", [D, EL], bf16, isOutput=False)
    t["wvT"] = nc.declare_dram_parameter("wvT", [D, EL], bf16, isOutput=False)
    t["woT"] = nc.declare_dram_parameter("woT", [EL, D], bf16, isOutput=False)
    t["cosT"] = nc.declare_dram_parameter("cosT", [NROT2, S], f32, isOutput=False)
    t["sinT"] = nc.declare_dram_parameter("sinT", [NROT2, S], f32, isOutput=False)
    t["masks"] = nc.declare_dram_parameter("masks", [P, 4, 512], f32, isOutput=False)
    t["out"] = nc.declare_dram_parameter("out", [S // NCORES, D], bf16, isOutput=True)
    t["qT_d"] = nc.dram_tensor("qT_d", [EL, S], bf16)
    t["pout_d"] = nc.dram_tensor("pout_d", [S, D], bf16)
    t["rs_d"] = nc.dram_tensor("rs_d", [S // NCORES, D], bf16)
    _emit(nc, t)
    nc.compile()
    _CACHE["nc"] = nc
    return nc


def _prep_inputs(hidden_states, Wq, Wk, Wv, Wo, attention_mask, position_ids):
    hs = np.asarray(hidden_states, np.float32).reshape(S, D)
    hsT = np.ascontiguousarray(hs.T).astype(ml_dtypes.bfloat16)

    pos = np.asarray(position_ids).reshape(S).astype(np.float32)
    inv = 10000.0 ** (-np.arange(0, ROT, 2, dtype=np.float32) / ROT)  # [32]
    ang = pos[:, None] * inv[None, :]                                  # [S, 32]
    cosT = np.ascontiguousarray(np.cos(ang).T).astype(np.float32)
    sinT = np.ascontiguousarray(np.sin(ang).T).astype(np.float32)

    am = np.asarray(attention_mask, np.float32).reshape(S, S)
    masks = np.empty((P, 4, 512), np.float32)
    for m in range(4):
        # transposed-score layout: mask[p, n] for k = m*128+p, q = n
        masks[:, m, :] = am[0:512, m * P:(m + 1) * P].T

    # within-head row permutation: even rot dims, odd rot dims, the rest
    perm1 = np.concatenate([np.arange(0, ROT, 2), np.arange(1, ROT, 2),
                            np.arange(ROT, HD)])
    perm = np.concatenate([perm1 + HD * j for j in range(HL)])

    Wq = np.asarray(Wq, np.float32)
    Wk = np.asarray(Wk, np.float32)
    Wv = np.asarray(Wv, np.float32)
    Wo = np.asarray(Wo, np.float32)
    scale = 1.0 / np.sqrt(np.float32(HD))

    in_maps = []
    for c in range(NCORES):
        rows = slice(c * EL, (c + 1) * EL)
        wq_c = Wq[rows][perm] * scale
        wk_c = Wk[rows][perm]
        wv_c = Wv[rows]
        in_maps.append({
            "hsT": hsT,
            "wqT": np.ascontiguousarray(wq_c.T).astype(ml_dtypes.bfloat16),
            "wkT": np.ascontiguousarray(wk_c.T).astype(ml_dtypes.bfloat16),
            "wvT": np.ascontiguousarray(wv_c.T).astype(ml_dtypes.bfloat16),
            "woT": np.ascontiguousarray(Wo[:, rows].T).astype(ml_dtypes.bfloat16),
            "cosT": cosT,
            "sinT": sinT,
            "masks": masks,
        })
    return in_maps


def run(inputs, trace=False):
    """Run on HW. Returns (full_output, BassKernelResults)."""
    nc = _build()
    in_maps = _prep_inputs(**inputs)
    res = run_bass_kernel_spmd(nc, in_maps, list(range(NCORES)), trace=trace)
    # RS chunk (start_r, nr): core c holds global rows
    # [start_r + (nr/8)*c, +nr/8), stored at rs offset start_r/8.
    full = np.empty((S, D), np.float32)
    for c in range(NCORES):
        shard = np.asarray(res.results[c]["out"]).astype(np.float32)
        for start_r, nr in RS_CHUNKS:
            o0 = start_r // NCORES
            w = nr // NCORES
            full[start_r + w * c:start_r + w * (c + 1)] = shard[o0:o0 + w]
    return full.reshape(B, S, D), res


def kernel(**inputs):
    full, _ = run(inputs, trace=False)
    return full
